# revision 1
# baseline (speedup 1.0000x reference)
"""Trainium2 Bass kernel: 8-head transformer encoder layer (B=8, S=1024,
D=300, Dh=512, H=8), data-parallel over batch across 8 NeuronCores.

Per core (one batch element):
  qT/kT = Wp @ x^T  (heads contiguous via host-side weight-row permute)
  v     = x @ Wp^T   (ones-augmented: column 64 of each head slice is 1.0,
                      so the AV matmul also produces the softmax denominator)
  per head: e = q k^T (PSUM) -> bn_stats var -> c = gamma/sqrt(var + D*eps)
            p = exp(c*e) (ACT; mean and beta_a drop by softmax shift
            invariance; post-LN rows are standardized so no max-subtraction)
            pT via PE transpose -> [heads; r]^T = v_aug^T-chunks @ pT
            aT = heads^T * broadcast(1/r)  (K=1 matmul broadcast)
  x1 = a @ WO ; x2 = LN(x1 + x) ; x2T via PE transpose
  h1T = relu(W1-as-lhsT @ x2T + b1) ; h2 = h1T-chunks @ W2
  out = LN(h2 + b2 + x2)
"""

import numpy as np

import concourse.bass as bass
import concourse.tile as tile
from concourse import bacc, mybir
from concourse.bass_utils import run_bass_kernel_spmd
from concourse.masks import make_identity

F32 = mybir.dt.float32
USE_F32R = False
FR = mybir.dt.float32r if USE_F32R else F32
AF = mybir.ActivationFunctionType

B, S, D, DH, H, DHD = 8, 1024, 300, 512, 8, 64
DF = 4 * D  # 1200
EPS = 1e-8
NCORES = 8

DP, DFP = 384, 1280  # D/DF zero-padded to 128 multiples (full-partition streams)
J_CHUNKS = [(0, 128), (128, 128), (256, 128)]
M_CHUNKS = [(i * 128, 128) for i in range(10)]
N_ST = S // 128  # 8 s-tiles
N_SH = S // 512  # 2 s-halves

TRACE = False
_cache = {}
_last_results = None


def _build_nc(dbg=False):
    nc = bacc.Bacc("TRN2", debug=False)

    xd = nc.dram_tensor("x", [S, D], F32, kind="ExternalInput").ap()
    xtd = nc.dram_tensor("xt", [DP, S], F32, kind="ExternalInput").ap()
    wqd = nc.dram_tensor("wq", [DP, DH], F32, kind="ExternalInput").ap()
    wkd = nc.dram_tensor("wk", [DP, DH], F32, kind="ExternalInput").ap()
    wvd = nc.dram_tensor("wv", [DP, DH], F32, kind="ExternalInput").ap()
    wod = nc.dram_tensor("wo", [DH, D], F32, kind="ExternalInput").ap()
    w1d = nc.dram_tensor("w1", [DP, DFP], F32, kind="ExternalInput").ap()
    w2d = nc.dram_tensor("w2", [DFP, D], F32, kind="ExternalInput").ap()
    fb1d = nc.dram_tensor("fb1", [1280, 1], F32, kind="ExternalInput").ap()
    fb2d = nc.dram_tensor("fb2", [D], F32, kind="ExternalInput").ap()
    gad = nc.dram_tensor("ga", [H, 1], F32, kind="ExternalInput").ap()
    lnd = nc.dram_tensor("ln", [4, 1], F32, kind="ExternalInput").ap()
    outd = nc.dram_tensor("out", [S, D], F32, kind="ExternalOutput").ap()
    if dbg:
        dqT = nc.dram_tensor("dqT", [DH, S], F32, kind="ExternalOutput").ap()
        dkT = nc.dram_tensor("dkT", [DH, S], F32, kind="ExternalOutput").ap()
        dv = nc.dram_tensor("dv", [S, DH], F32, kind="ExternalOutput").ap()
        dx2 = nc.dram_tensor("dx2", [S, D], F32, kind="ExternalOutput").ap()
        dh1 = nc.dram_tensor("dh1", [DF, S], F32, kind="ExternalOutput").ap()
        dc8 = nc.dram_tensor("dc8", [H, 128, N_ST], F32, kind="ExternalOutput").ap()

    def bcr(ap):
        return ap.bitcast(FR) if USE_F32R else ap

    with tile.TileContext(nc) as tc:
        with (
            tc.tile_pool(name="wts", bufs=1) as wts,
            tc.tile_pool(name="work", bufs=1) as work,
            tc.tile_pool(name="sm", bufs=8) as sm,
            tc.tile_pool(name="ps", bufs=1, space="PSUM") as ps,
        ):
            # ---------------- constant / weight loads ----------------
            ident = wts.tile([128, 128], F32, tag="ident")
            make_identity(nc, ident)
            identr = ident

            ones1 = wts.tile([1, 128], F32, tag="ones1")
            nc.vector.memset(ones1, 1.0)
            dummy = wts.tile([128, 1], F32, tag="dummy")
            nc.vector.memset(dummy, 1.0)
            dsink = wts.tile([128, 1], F32, tag="dsink")

            eps_a = wts.tile([128, 1], F32, tag="eps_a")  # D*EPS (score LN)
            nc.vector.memset(eps_a, D * EPS)
            eps_l = wts.tile([128, 1], F32, tag="eps_l")  # EPS (x LNs)
            nc.vector.memset(eps_l, EPS)

            def bcast_load(src_ap, shape, tag):
                t = wts.tile(shape, F32, tag=tag)
                nc.sync.dma_start(out=t, in_=src_ap.to_broadcast(shape))
                return t

            ga_bc = [bcast_load(gad[h : h + 1, :], [128, 1], f"ga{h}") for h in range(H)]
            g1_bc = bcast_load(lnd[0:1, :], [128, 1], "g1")
            b1_bc = bcast_load(lnd[1:2, :], [128, 1], "b1")
            g2_bc = bcast_load(lnd[2:3, :], [128, 1], "g2")
            b2_bc = bcast_load(lnd[3:4, :], [128, 1], "b2")
            fb2_bc = wts.tile([128, D], F32, tag="fb2")
            nc.sync.dma_start(
                out=fb2_bc,
                in_=bass.AP(tensor=fb2d.tensor, offset=fb2d.offset,
                            ap=[[0, 128]] + list(fb2d.ap)),
            )
            fb1_sb = []
            for mt, (m0, msz) in enumerate(M_CHUNKS):
                t = wts.tile([128, 1], F32, tag=f"fb1_{mt}")
                nc.sync.dma_start(out=t[:msz, :], in_=fb1d[m0 : m0 + msz, :])
                fb1_sb.append(t)

            # x natural: [128, 8, 300] (partition = s % 128)
            x_sb = wts.tile([128, N_ST, D], F32, tag="x")
            nc.sync.dma_start(out=x_sb, in_=xd.rearrange("(n p) d -> p n d", p=128))

            def chunked_load(src, width, tag):
                tiles = []
                for jc, (j0, jn) in enumerate(J_CHUNKS):
                    t = wts.tile([128, width], FR, tag=f"{tag}{jc}")
                    nc.sync.dma_start(out=t[:jn, :], in_=bcr(src[j0 : j0 + jn, :]))
                    tiles.append(t)
                return tiles

            xt_sb = chunked_load(xtd, S, "xt")    # [300, 1024] in 3 chunks
            wq_sb = chunked_load(wqd, DH, "wq")   # [300, 512]
            wk_sb = chunked_load(wkd, DH, "wk")
            wv_sb = chunked_load(wvd, DH, "wv")
            w1_sb = chunked_load(w1d, DFP, "w1")   # [300, 1200]

            wo_sb = []
            for it in range(4):
                t = wts.tile([128, D], FR, tag=f"wo{it}")
                nc.sync.dma_start(out=t, in_=bcr(wod[it * 128 : (it + 1) * 128, :]))
                wo_sb.append(t)
            w2_sb = []
            for mt, (m0, msz) in enumerate(M_CHUNKS):
                t = wts.tile([128, D], FR, tag=f"w2_{mt}")
                nc.sync.dma_start(out=t[:msz, :], in_=bcr(w2d[m0 : m0 + msz, :]))
                w2_sb.append(t)

            # ---------------- phase 1: projections ----------------
            qT = [work.tile([128, S], FR, tag="big4k", bufs=14, name=f"qT{i}") for i in range(4)]
            kT = [work.tile([128, S], FR, tag="big4k", bufs=14, name=f"kT{i}") for i in range(4)]
            v_sb = [work.tile([128, H, DHD + 1], F32, tag="v2k", bufs=10, name=f"v{i}") for i in range(N_ST)]

            for dst, w in ((qT, wq_sb), (kT, wk_sb)):
                for dt in range(4):
                    for sh in range(N_SH):
                        pp = ps.tile([128, 512], F32, tag="e", bufs=5)
                        for jc, (j0, jn) in enumerate(J_CHUNKS):
                            nc.tensor.matmul(
                                pp,
                                lhsT=w[jc][:jn, dt * 128 : (dt + 1) * 128],
                                rhs=xt_sb[jc][:jn, sh * 512 : (sh + 1) * 512],
                                start=(jc == 0),
                                stop=(jc == 2),
                            )
                        nc.vector.tensor_copy(out=dst[dt][:, sh * 512 : (sh + 1) * 512], in_=pp)
            G_ps = ps.tile([64, H, DHD], F32, tag="pt", bufs=3)
            nc.vector.memset(G_ps, 0.0)
            for st in range(N_ST):
                pp = ps.tile([128, 512], F32, tag="e", bufs=5)
                for jc, (j0, jn) in enumerate(J_CHUNKS):
                    nc.tensor.matmul(
                        pp,
                        lhsT=xt_sb[jc][:jn, st * 128 : (st + 1) * 128],
                        rhs=wv_sb[jc][:jn, :],
                        start=(jc == 0),
                        stop=(jc == 2),
                    )
                nc.vector.tensor_copy(
                    out=v_sb[st][:, :, 0:DHD],
                    in_=pp.rearrange("p (h d) -> p h d", h=H),
                )
                nc.vector.memset(v_sb[st][:, :, DHD : DHD + 1], 1.0)
                # k in natural [t, d] layout, for G_h = sum_t k_t k_t^T
                pk = ps.tile([128, 512], F32, tag="e", bufs=5)
                for jc, (j0, jn) in enumerate(J_CHUNKS):
                    nc.tensor.matmul(
                        pk,
                        lhsT=xt_sb[jc][:jn, st * 128 : (st + 1) * 128],
                        rhs=wk_sb[jc][:jn, :],
                        start=(jc == 0),
                        stop=(jc == 2),
                    )
                kn = work.tile([128, 512], F32, tag="v2k", bufs=10, name="kn")
                nc.vector.tensor_copy(out=kn, in_=pk)
                for h in range(H):
                    nc.tensor.matmul(
                        G_ps[:, h, :],
                        lhsT=kn[:, h * DHD : (h + 1) * DHD],
                        rhs=kn[:, h * DHD : (h + 1) * DHD],
                        start=False,
                        stop=(st == N_ST - 1),
                        skip_group_check=True,
                    )

            if dbg:
                for i in range(4):
                    nc.sync.dma_start(out=dqT[i * 128 : (i + 1) * 128, :], in_=qT[i].bitcast(F32))
                    nc.sync.dma_start(out=dkT[i * 128 : (i + 1) * 128, :], in_=kT[i].bitcast(F32))
                for i in range(N_ST):
                    nc.sync.dma_start(out=dv[i * 128 : (i + 1) * 128, :],
                                      in_=v_sb[i][:, :, 0:DHD])

            # ---------------- phase 2: attention ----------------
            # Analytic score stats: sum_t e = q . ksum, sum_t e^2 = q^T G q,
            # so the softmax scale c is ready before scores ever run and the
            # scores->exp chain has no cross-engine stats dependency.
            aT = [work.tile([128, S], FR, tag="big4k", bufs=14, name=f"aT{i}") for i in range(4)]

            G_sb = wts.tile([128, H, DHD], F32, tag="gsb")
            nc.vector.tensor_copy(out=G_sb[0:64, :, :], in_=G_ps)
            nc.sync.dma_start(out=G_sb[64:128, :, :], in_=G_sb[0:64, :, :])
            ksum_t = wts.tile([128, H], F32, tag="ksum")
            for h in range(H):
                hp = (h % 2) * 64
                nc.vector.reduce_sum(
                    out=ksum_t[hp : hp + 64, h : h + 1],
                    in_=kT[h // 2][hp : hp + 64, :],
                    axis=mybir.AxisListType.X,
                )
            c8_t = [None] * H
            for hq in range(4):
                sums2 = [ps.tile([128, N_ST, 2], F32, tag="e", bufs=5, name=f"sm{j}")
                         for j in range(2)]
                for sh in range(N_SH):
                    # two heads' y matmuls adjacent: half-BW streams overlap
                    y2 = [ps.tile([128, 512], F32, tag="pt", bufs=3, name=f"y{j}")
                          for j in range(2)]
                    for j in range(2):
                        hp = j * 64
                        nc.tensor.matmul(
                            y2[j][hp : hp + 64, :],
                            lhsT=G_sb[hp : hp + 64, hq * 2 + j, :],
                            rhs=qT[hq][hp : hp + 64, sh * 512 : (sh + 1) * 512],
                            start=True,
                            stop=True,
                        )
                    z_sb = sm.tile([128, 512], F32, tag="z", bufs=2)
                    for j in range(2):
                        hp = j * 64
                        nc.vector.tensor_tensor(
                            out=z_sb[hp : hp + 64, :],
                            in0=qT[hq][hp : hp + 64, sh * 512 : (sh + 1) * 512],
                            in1=y2[j][hp : hp + 64, :],
                            op=mybir.AluOpType.mult,
                        )
                    for st4 in range(4):
                        st = sh * 4 + st4
                        for j in range(2):
                            hp = j * 64
                            h = hq * 2 + j
                            nc.tensor.matmul(
                                sums2[j][:, st, 0:1],
                                lhsT=qT[hq][hp : hp + 64, st * 128 : (st + 1) * 128],
                                rhs=ksum_t[hp : hp + 64, h : h + 1],
                                start=True,
                                stop=True,
                            )
                        for j in range(2):
                            hp = j * 64
                            nc.tensor.matmul(
                                sums2[j][:, st, 1:2],
                                lhsT=z_sb[hp : hp + 64, st4 * 128 : (st4 + 1) * 128],
                                rhs=dummy[hp : hp + 64, :],
                                start=True,
                                stop=True,
                            )
                for j in range(2):
                    h = hq * 2 + j
                    hp = j * 64
                    sums_ps = sums2[j]
                    sums_sb = sm.tile([128, N_ST, 2], F32, tag="sums", bufs=2)
                    nc.vector.tensor_copy(out=sums_sb, in_=sums_ps)
                    m2 = sm.tile([128, N_ST], F32, tag="m2", bufs=2)
                    nc.vector.tensor_tensor(
                        out=m2, in0=sums_sb[:, :, 0], in1=sums_sb[:, :, 0],
                        op=mybir.AluOpType.mult,
                    )
                    nc.vector.tensor_scalar_mul(m2, m2, -1.0 / S)
                    nc.vector.tensor_tensor(
                        out=m2, in0=m2, in1=sums_sb[:, :, 1], op=mybir.AluOpType.add
                    )
                    # c = gamma / sqrt(M2/(S-1) + D*eps)
                    c8 = sm.tile([128, N_ST], F32, tag=f"c8_{h}", bufs=1)
                    nc.scalar.activation(
                        out=c8, in_=m2, func=AF.Sqrt, bias=eps_a, scale=1.0 / (S - 1)
                    )
                    nc.vector.reciprocal(out=c8, in_=c8)
                    nc.vector.tensor_scalar_mul(c8, c8, ga_bc[h])
                    if dbg:
                        nc.sync.dma_start(out=dc8[h], in_=c8)
                    c8_t[h] = c8

            pending = []

            def flush_pending():
                # deferred per-head normalization: by now the rrow reciprocal
                # has long finished, so the rbc matmul never stalls the PE
                while pending:
                    dst_hq, dst_sh, j, av_ps, rrow = pending.pop(0)
                    hp = j * 64
                    rbc_ps = ps.tile([128, 512], F32, tag="e", bufs=5)
                    nc.tensor.matmul(rbc_ps, lhsT=ones1, rhs=rrow, start=True, stop=True)
                    rbc_sb = sm.tile([128, 512], F32, tag="rbc", bufs=2)
                    nc.vector.tensor_copy(out=rbc_sb, in_=rbc_ps)
                    nc.vector.tensor_tensor(
                        out=aT[dst_hq][hp : hp + 64, dst_sh * 512 : (dst_sh + 1) * 512],
                        in0=av_ps[0:DHD, :],
                        in1=rbc_sb[0:DHD, :],
                        op=mybir.AluOpType.mult,
                    )

            for hq in range(4):
                qt_t = qT[hq]
                kt_t = kT[hq]
                for sh in range(N_SH):
                    pT2 = [
                        work.tile([128, 8, 512], F32, tag="pt16k", bufs=2, name=f"pT{j}")
                        for j in range(2)
                    ]
                    for stq in range(4):
                        st = sh * 4 + stq
                        # all 4 score matmuls adjacent: the two heads sit in
                        # distinct PE row groups and their half-bandwidth
                        # 64-partition rhs streams overlap
                        e_t = [[None, None], [None, None]]
                        for th in range(2):
                            for j in range(2):
                                hp = j * 64
                                eh = ps.tile([128, 512], F32, tag="e", bufs=5,
                                             name=f"eh{j}{th}")
                                e_t[j][th] = eh
                                nc.tensor.matmul(
                                    eh,
                                    lhsT=qt_t[hp : hp + 64, st * 128 : (st + 1) * 128],
                                    rhs=kt_t[hp : hp + 64, th * 512 : (th + 1) * 512],
                                    start=True,
                                    stop=True,
                                )
                        p2 = [None, None]
                        for j in range(2):
                            c8 = c8_t[hq * 2 + j]
                            p_sb = work.tile([128, S], F32, tag="big4k", bufs=14,
                                             name=f"p{j}")
                            p2[j] = p_sb
                            for th in range(2):
                                nc.scalar.activation(
                                    out=p_sb[:, th * 512 : (th + 1) * 512],
                                    in_=e_t[j][th], func=AF.Exp, bias=0.0,
                                    scale=c8[:, st : st + 1],
                                )
                        for j in range(2):
                            for half in range(2):
                                pt_ps = ps.tile([128, 4, 128], F32, tag="pt", bufs=3)
                                for k in range(4):
                                    tj = half * 4 + k
                                    nc.tensor.transpose(
                                        pt_ps[:, k, :],
                                        p2[j][:, tj * 128 : (tj + 1) * 128],
                                        identr,
                                    )
                                nc.vector.tensor_copy(
                                    out=pT2[j][:, half * 4 : half * 4 + 4,
                                           stq * 128 : (stq + 1) * 128],
                                    in_=pt_ps,
                                )
                    av_list = []
                    for j in range(2):
                        h = hq * 2 + j
                        av_ps = ps.tile([DHD + 1, 512], F32, tag="pt", bufs=3)
                        for tj in range(8):
                            nc.tensor.matmul(
                                av_ps,
                                lhsT=v_sb[tj][:, h, :],
                                rhs=pT2[j][:, tj, :],
                                start=(tj == 0),
                                stop=(tj == 7),
                            )
                        rrow = sm.tile([1, 512], F32, tag="rrow", bufs=2)
                        nc.vector.reciprocal(out=rrow, in_=av_ps[DHD : DHD + 1, :])
                        av_list.append((j, av_ps, rrow))
                    for j, av_ps, rrow in av_list:
                        pending.append((hq, sh, j, av_ps, rrow))
                    flush_pending()

            # ---------------- phase 3: WO + residual + LN1 ----------------
            x2_sb = [work.tile([128, D], F32, tag="v2k", bufs=10, name=f"x2_{i}") for i in range(N_ST)]
            x2T = [work.tile([128, S], FR, tag="big4k", bufs=14, name=f"x2T{i}") for i in range(3)]
            nc.vector.memset(x2T[2], 0.0)
            LCORR = float(D) / float(D - 1)

            def layer_norm(dst, src_ps, res_tiles, g_bc, b_bc):
                xr = sm.tile([128, D], F32, tag="xr", bufs=2)
                nc.vector.tensor_add(xr, src_ps, res_tiles[0])
                for rt in res_tiles[1:]:
                    nc.vector.tensor_add(xr, xr, rt)
                stats = sm.tile([128, 6], F32, tag="lstats", bufs=4)
                nc.vector.bn_stats(out=stats, in_=xr)
                mv = sm.tile([128, 2], F32, tag="lmv", bufs=4)
                nc.vector.bn_aggr(out=mv, in_=stats)
                sd = sm.tile([128, 1], F32, tag="lsd", bufs=4)
                nc.scalar.activation(
                    out=sd, in_=mv[:, 1:2], func=AF.Sqrt, bias=eps_l, scale=LCORR
                )
                rstd = sm.tile([128, 1], F32, tag="lrstd", bufs=4)
                nc.vector.reciprocal(out=rstd, in_=sd)
                grstd = sm.tile([128, 1], F32, tag="lgr", bufs=4)
                nc.vector.tensor_mul(grstd, rstd, g_bc)
                nc.vector.tensor_scalar(
                    out=dst,
                    in0=xr,
                    scalar1=mv[:, 0:1],
                    scalar2=grstd,
                    op0=mybir.AluOpType.subtract,
                    op1=mybir.AluOpType.mult,
                )
                nc.vector.tensor_scalar_add(dst, dst, b_bc)

            for st in range(N_ST):
                x1_ps = ps.tile([128, D], F32, tag="e", bufs=5)
                for it in range(4):
                    nc.tensor.matmul(
                        x1_ps,
                        lhsT=aT[it][:, st * 128 : (st + 1) * 128],
                        rhs=wo_sb[it],
                        start=(it == 0),
                        stop=(it == 3),
                    )
                layer_norm(x2_sb[st], x1_ps, [x_sb[:, st, :]], g1_bc, b1_bc)
                xt_ps = ps.tile([128, 4, 128], F32, tag="pt", bufs=3)
                for jc, (j0, jn) in enumerate([(0, 128), (128, 128), (256, 44)]):
                    nc.tensor.transpose(
                        xt_ps[:jn, jc, :], x2_sb[st][:, j0 : j0 + jn], ident
                    )
                for jc, (j0, jn) in enumerate([(0, 128), (128, 128), (256, 44)]):
                    nc.vector.tensor_copy(
                        out=x2T[jc][:jn, st * 128 : (st + 1) * 128],
                        in_=xt_ps[:jn, jc, :],
                    )

            # ---------------- phase 4: FFN + LN2 ----------------
            h1T = [work.tile([128, S], FR, tag="big4k", bufs=14, name=f"h1T{i}") for i in range(10)]
            for mt, (m0, msz) in enumerate(M_CHUNKS):
                for sh in range(N_SH):
                    h1_ps = ps.tile([128, 512], F32, tag="e", bufs=5)
                    for jc, (j0, jn) in enumerate(J_CHUNKS):
                        nc.tensor.matmul(
                            h1_ps[:msz, :],
                            lhsT=w1_sb[jc][:jn, m0 : m0 + msz],
                            rhs=x2T[jc][:jn, sh * 512 : (sh + 1) * 512],
                            start=(jc == 0),
                            stop=(jc == 2),
                        )
                    nc.scalar.activation(
                        out=h1T[mt][:msz, sh * 512 : (sh + 1) * 512],
                        in_=h1_ps[:msz, :],
                        func=AF.Relu,
                        bias=fb1_sb[mt][:msz, :],
                        scale=1.0,
                    )
            if dbg:
                for i in range(N_ST):
                    nc.sync.dma_start(out=dx2[i * 128 : (i + 1) * 128, :], in_=x2_sb[i])
                for mt, (m0, msz) in enumerate(M_CHUNKS):
                    mz = min(msz, DF - m0)
                    nc.sync.dma_start(out=dh1[m0 : m0 + mz, :], in_=h1T[mt][:mz, :].bitcast(F32))
            for st in range(N_ST):
                h2_ps = ps.tile([128, D], F32, tag="e", bufs=5)
                for mt, (m0, msz) in enumerate(M_CHUNKS):
                    nc.tensor.matmul(
                        h2_ps,
                        lhsT=h1T[mt][:msz, st * 128 : (st + 1) * 128],
                        rhs=w2_sb[mt][:msz, :],
                        start=(mt == 0),
                        stop=(mt == 9),
                    )
                o_sb = sm.tile([128, D], F32, tag="o", bufs=2)
                layer_norm(o_sb, h2_ps, [fb2_bc, x2_sb[st]], g2_bc, b2_bc)
                nc.sync.dma_start(out=outd[st * 128 : (st + 1) * 128, :], in_=o_sb)

    nc.compile()
    return nc


def _get_nc():
    if "nc" not in _cache:
        _cache["nc"] = _build_nc()
    return _cache["nc"]


def kernel(x, WQ, WK, WV, WO, W1, b1, W2, b2, gamma_a, beta_a,
           gamma1, beta1, gamma2, beta2):
    global _last_results
    f = np.float32
    x = np.asarray(x, f)

    def perm(W):
        # head h -> contiguous rows [h*64, (h+1)*64)
        return np.asarray(W, f).reshape(DHD, H, D).transpose(1, 0, 2).reshape(DH, D)

    def padr(a, rows, cols=None):
        out = np.zeros((rows, cols or a.shape[1]), f)
        out[: a.shape[0], : a.shape[1]] = a
        return out

    wq_t = padr(perm(WQ).T, DP)
    wk_t = padr(perm(WK).T, DP)
    wv_t = padr(perm(WV).T, DP)
    wo = np.ascontiguousarray(np.asarray(WO, f))
    w1 = padr(np.asarray(W1, f), DP, DFP)
    w2 = padr(np.asarray(W2, f), DFP)
    fb1 = np.zeros((1280, 1), f)
    fb1[:DF, 0] = np.asarray(b1, f)
    fb2 = np.ascontiguousarray(np.asarray(b2, f))
    # beta_a drops out of softmax (per-row constant shift); the 1/sqrt(D)
    # score scale cancels inside the score LayerNorm: softmax(g*LN(e/sqrt(D)))
    # == softmax(g/sqrt(var(e) + D*eps) * e), so gamma is used unscaled and
    # D*eps replaces eps on-device.
    ga = np.ascontiguousarray(np.asarray(gamma_a, f).reshape(H, 1))
    ln = np.array(
        [np.asarray(gamma1, f), np.asarray(beta1, f),
         np.asarray(gamma2, f), np.asarray(beta2, f)], f
    ).reshape(4, 1)

    shared = {"wq": wq_t, "wk": wk_t, "wv": wv_t, "wo": wo, "w1": w1, "w2": w2,
              "fb1": fb1, "fb2": fb2, "ga": ga, "ln": ln}
    in_maps = []
    for b in range(B):
        xb = np.ascontiguousarray(x[b])
        in_maps.append({"x": xb, "xt": padr(np.ascontiguousarray(xb.T), DP), **shared})

    nc = _get_nc()
    res = run_bass_kernel_spmd(nc, in_maps, core_ids=list(range(NCORES)), trace=TRACE)
    _last_results = res
    return np.stack([res.results[b]["out"] for b in range(B)], axis=0)



# revision 16
# speedup vs baseline: 1.7930x; 1.7930x over previous
"""Trainium2 Bass kernel: 8-head transformer encoder layer (B=8, S=1024,
D=300, Dh=512, H=8), data-parallel over batch across 8 NeuronCores.

Per core (one batch element):
  qT/kT = Wp @ x^T  (heads contiguous via host-side weight-row permute)
  v     = x @ Wp^T   (ones-augmented: column 64 of each head slice is 1.0,
                      so the AV matmul also produces the softmax denominator)
  per head: e = q k^T (PSUM) -> bn_stats var -> c = gamma/sqrt(var + D*eps)
            p = exp(c*e) (ACT; mean and beta_a drop by softmax shift
            invariance; post-LN rows are standardized so no max-subtraction)
            pT via PE transpose -> [heads; r]^T = v_aug^T-chunks @ pT
            aT = heads^T * broadcast(1/r)  (K=1 matmul broadcast)
  x1 = a @ WO ; x2 = LN(x1 + x) ; x2T via PE transpose
  h1T = relu(W1-as-lhsT @ x2T + b1) ; h2 = h1T-chunks @ W2
  out = LN(h2 + b2 + x2)
"""

import numpy as np

import concourse.bass as bass
import concourse.tile as tile
from concourse import bacc, mybir
from concourse.bass_utils import run_bass_kernel_spmd
from concourse.masks import make_identity

F32 = mybir.dt.float32
BF16 = mybir.dt.bfloat16
FR = BF16  # matmul operand dtype: 1 cycle/row on the PE vs 4 for fp32
AF = mybir.ActivationFunctionType

B, S, D, DH, H, DHD = 8, 1024, 300, 512, 8, 64
DF = 4 * D  # 1200
EPS = 1e-8
NCORES = 8

DP, DFP = 384, 1280  # D/DF zero-padded to 128 multiples (full-partition streams)
J_CHUNKS = [(0, 128), (128, 128), (256, 128)]
M_CHUNKS = [(i * 128, 128) for i in range(10)]
N_ST = S // 128  # 8 s-tiles
N_SH = S // 512  # 2 s-halves

TRACE = False
_cache = {}
_last_results = None


def _build_nc(dbg=False):
    nc = bacc.Bacc("TRN2", debug=False)

    xd = nc.dram_tensor("x", [S, D], F32, kind="ExternalInput").ap()
    xtd = nc.dram_tensor("xt", [DP, S], FR, kind="ExternalInput").ap()
    wqd = nc.dram_tensor("wq", [DP, DH], FR, kind="ExternalInput").ap()
    wkd = nc.dram_tensor("wk", [DP, DH], FR, kind="ExternalInput").ap()
    wvd = nc.dram_tensor("wv", [DP, DH], FR, kind="ExternalInput").ap()
    wod = nc.dram_tensor("wo", [DH, D], FR, kind="ExternalInput").ap()
    w1d = nc.dram_tensor("w1", [DP, DFP], FR, kind="ExternalInput").ap()
    w2d = nc.dram_tensor("w2", [DFP, D], FR, kind="ExternalInput").ap()
    fb1d = nc.dram_tensor("fb1", [1280, 1], F32, kind="ExternalInput").ap()
    fb2d = nc.dram_tensor("fb2", [D], F32, kind="ExternalInput").ap()
    gad = nc.dram_tensor("ga", [H, 1], F32, kind="ExternalInput").ap()
    lnd = nc.dram_tensor("ln", [4, 1], F32, kind="ExternalInput").ap()
    outd = nc.dram_tensor("out", [S, D], F32, kind="ExternalOutput").ap()
    if dbg:
        dqT = nc.dram_tensor("dqT", [DH, S], F32, kind="ExternalOutput").ap()
        dkT = nc.dram_tensor("dkT", [DH, S], F32, kind="ExternalOutput").ap()
        dv = nc.dram_tensor("dv", [S, DH], F32, kind="ExternalOutput").ap()
        dx2 = nc.dram_tensor("dx2", [S, D], F32, kind="ExternalOutput").ap()
        dh1 = nc.dram_tensor("dh1", [DF, S], F32, kind="ExternalOutput").ap()
        dc8 = nc.dram_tensor("dc8", [H, 128, N_ST], F32, kind="ExternalOutput").ap()

    def bcr(ap):
        return ap

    with tile.TileContext(nc) as tc:
        with (
            tc.tile_pool(name="wts", bufs=1) as wts,
            tc.tile_pool(name="work", bufs=1) as work,
            tc.tile_pool(name="sm", bufs=8) as sm,
            tc.tile_pool(name="ps", bufs=1, space="PSUM") as ps,
        ):
            # ---------------- constant / weight loads ----------------
            ident = wts.tile([128, 128], F32, tag="ident")
            make_identity(nc, ident)
            identr = wts.tile([128, 128], FR, tag="identr")
            nc.vector.tensor_copy(out=identr, in_=ident)

            ones1 = wts.tile([1, 128], FR, tag="ones1")
            nc.vector.memset(ones1, 1.0)
            dummy = wts.tile([128, 1], FR, tag="dummy")
            nc.vector.memset(dummy, 1.0)
            dsink = wts.tile([128, 1], F32, tag="dsink")

            eps_a = wts.tile([128, 1], F32, tag="eps_a")  # D*EPS (score LN)
            nc.vector.memset(eps_a, D * EPS)
            eps_l = wts.tile([128, 1], F32, tag="eps_l")  # EPS (x LNs)
            nc.vector.memset(eps_l, EPS)

            def bcast_load(src_ap, shape, tag):
                t = wts.tile(shape, F32, tag=tag)
                nc.sync.dma_start(out=t, in_=src_ap.to_broadcast(shape))
                return t

            ga_bc = [bcast_load(gad[h : h + 1, :], [128, 1], f"ga{h}") for h in range(H)]
            g1_bc = bcast_load(lnd[0:1, :], [128, 1], "g1")
            b1_bc = bcast_load(lnd[1:2, :], [128, 1], "b1")
            g2_bc = bcast_load(lnd[2:3, :], [128, 1], "g2")
            b2_bc = bcast_load(lnd[3:4, :], [128, 1], "b2")
            fb2_bc = wts.tile([128, D], F32, tag="fb2")
            nc.sync.dma_start(
                out=fb2_bc,
                in_=bass.AP(tensor=fb2d.tensor, offset=fb2d.offset,
                            ap=[[0, 128]] + list(fb2d.ap)),
            )
            fb1_sb = []
            for mt, (m0, msz) in enumerate(M_CHUNKS):
                t = wts.tile([128, 1], F32, tag=f"fb1_{mt}")
                nc.sync.dma_start(out=t[:msz, :], in_=fb1d[m0 : m0 + msz, :])
                fb1_sb.append(t)

            # x natural: [128, 8, 300] (partition = s % 128)
            x_sb = wts.tile([128, N_ST, D], F32, tag="x")
            nc.sync.dma_start(out=x_sb, in_=xd.rearrange("(n p) d -> p n d", p=128))

            def chunked_load(src, width, tag):
                tiles = []
                for jc, (j0, jn) in enumerate(J_CHUNKS):
                    t = wts.tile([128, width], FR, tag=f"{tag}{jc}")
                    nc.sync.dma_start(out=t[:jn, :], in_=bcr(src[j0 : j0 + jn, :]))
                    tiles.append(t)
                return tiles

            xt_sb = chunked_load(xtd, S, "xt")    # [300, 1024] in 3 chunks
            wq_sb = chunked_load(wqd, DH, "wq")   # [300, 512]
            wk_sb = chunked_load(wkd, DH, "wk")
            wv_sb = chunked_load(wvd, DH, "wv")
            w1_sb = chunked_load(w1d, DFP, "w1")   # [300, 1200]

            wo_sb = []
            for it in range(4):
                t = wts.tile([128, D], FR, tag=f"wo{it}")
                nc.sync.dma_start(out=t, in_=bcr(wod[it * 128 : (it + 1) * 128, :]))
                wo_sb.append(t)
            w2_sb = []
            for mt, (m0, msz) in enumerate(M_CHUNKS):
                t = wts.tile([128, D], FR, tag=f"w2_{mt}")
                nc.sync.dma_start(out=t[:msz, :], in_=bcr(w2d[m0 : m0 + msz, :]))
                w2_sb.append(t)

            # ---------------- phase 1: projections ----------------
            qT = [work.tile([128, S], FR, tag="big4k", bufs=14, name=f"qT{i}") for i in range(4)]
            kT = [work.tile([128, S], FR, tag="big4k", bufs=14, name=f"kT{i}") for i in range(4)]
            v_sb = [work.tile([128, H, DHD + 1], FR, tag="v2k", bufs=10, name=f"v{i}") for i in range(N_ST)]

            for dst, w in ((qT, wq_sb), (kT, wk_sb)):
                for dt in range(4):
                    for sh in range(N_SH):
                        pp = ps.tile([128, 512], F32, tag="e", bufs=4)
                        for jc, (j0, jn) in enumerate(J_CHUNKS):
                            nc.tensor.matmul(
                                pp,
                                lhsT=w[jc][:jn, dt * 128 : (dt + 1) * 128],
                                rhs=xt_sb[jc][:jn, sh * 512 : (sh + 1) * 512],
                                start=(jc == 0),
                                stop=(jc == 2),
                            )
                        nc.vector.tensor_copy(out=dst[dt][:, sh * 512 : (sh + 1) * 512], in_=pp)
            G_ps = ps.tile([64, H, DHD], F32, tag="pt", bufs=2)
            nc.vector.memset(G_ps, 0.0)
            for st in range(N_ST):
                pp = ps.tile([128, 512], F32, tag="e", bufs=4)
                for jc, (j0, jn) in enumerate(J_CHUNKS):
                    nc.tensor.matmul(
                        pp,
                        lhsT=xt_sb[jc][:jn, st * 128 : (st + 1) * 128],
                        rhs=wv_sb[jc][:jn, :],
                        start=(jc == 0),
                        stop=(jc == 2),
                    )
                nc.vector.tensor_copy(
                    out=v_sb[st][:, :, 0:DHD],
                    in_=pp.rearrange("p (h d) -> p h d", h=H),
                )
                nc.vector.memset(v_sb[st][:, :, DHD : DHD + 1], 1.0)
                # k in natural [t, d] layout, for G_h = sum_t k_t k_t^T
                pk = ps.tile([128, 512], F32, tag="e", bufs=4)
                for jc, (j0, jn) in enumerate(J_CHUNKS):
                    nc.tensor.matmul(
                        pk,
                        lhsT=xt_sb[jc][:jn, st * 128 : (st + 1) * 128],
                        rhs=wk_sb[jc][:jn, :],
                        start=(jc == 0),
                        stop=(jc == 2),
                    )
                kn = work.tile([128, 512], FR, tag="v2k", bufs=10, name="kn")
                nc.vector.tensor_copy(out=kn, in_=pk)
                for h in range(H):
                    nc.tensor.matmul(
                        G_ps[:, h, :],
                        lhsT=kn[:, h * DHD : (h + 1) * DHD],
                        rhs=kn[:, h * DHD : (h + 1) * DHD],
                        start=False,
                        stop=(st == N_ST - 1),
                        skip_group_check=True,
                    )

            if dbg:
                for i in range(4):
                    nc.sync.dma_start(out=dqT[i * 128 : (i + 1) * 128, :], in_=qT[i].bitcast(F32))
                    nc.sync.dma_start(out=dkT[i * 128 : (i + 1) * 128, :], in_=kT[i].bitcast(F32))
                for i in range(N_ST):
                    nc.sync.dma_start(out=dv[i * 128 : (i + 1) * 128, :],
                                      in_=v_sb[i][:, :, 0:DHD])

            # ---------------- phase 2: attention ----------------
            # Analytic score stats: sum_t e = q . ksum, sum_t e^2 = q^T G q,
            # so the softmax scale c is ready before scores ever run and the
            # scores->exp chain has no cross-engine stats dependency.
            aT = [work.tile([128, S], FR, tag="big4k", bufs=14, name=f"aT{i}") for i in range(4)]

            G_sb = wts.tile([128, H, DHD], FR, tag="gsb")
            nc.vector.tensor_copy(out=G_sb[0:64, :, :], in_=G_ps)
            nc.sync.dma_start(out=G_sb[64:128, :, :], in_=G_sb[0:64, :, :])
            ksum_t = wts.tile([128, H], FR, tag="ksum")
            for h in range(H):
                hp = (h % 2) * 64
                with nc.allow_low_precision(reason="ksum feeds (sum e)^2/S, ~0.1% of M2"):
                    nc.vector.reduce_sum(
                        out=ksum_t[hp : hp + 64, h : h + 1],
                        in_=kT[h // 2][hp : hp + 64, :],
                        axis=mybir.AxisListType.X,
                    )
            c8_t = [None] * H
            for hq in range(4):
                sums2 = [ps.tile([128, N_ST, 2], F32, tag="e", bufs=4, name=f"sm{j}")
                         for j in range(2)]
                for sh in range(N_SH):
                    # two heads' y matmuls adjacent: half-BW streams overlap
                    y2 = [ps.tile([128, 512], F32, tag="pt", bufs=2, name=f"y{j}")
                          for j in range(2)]
                    for j in range(2):
                        hp = j * 64
                        nc.tensor.matmul(
                            y2[j][hp : hp + 64, :],
                            lhsT=G_sb[hp : hp + 64, hq * 2 + j, :],
                            rhs=qT[hq][hp : hp + 64, sh * 512 : (sh + 1) * 512],
                            start=True,
                            stop=True,
                        )
                    z_sb = sm.tile([128, 512], FR, tag="z", bufs=2)
                    for j in range(2):
                        hp = j * 64
                        nc.vector.tensor_tensor(
                            out=z_sb[hp : hp + 64, :],
                            in0=qT[hq][hp : hp + 64, sh * 512 : (sh + 1) * 512],
                            in1=y2[j][hp : hp + 64, :],
                            op=mybir.AluOpType.mult,
                        )
                    for st4 in range(4):
                        st = sh * 4 + st4
                        for j in range(2):
                            hp = j * 64
                            h = hq * 2 + j
                            nc.tensor.matmul(
                                sums2[j][:, st, 0:1],
                                lhsT=qT[hq][hp : hp + 64, st * 128 : (st + 1) * 128],
                                rhs=ksum_t[hp : hp + 64, h : h + 1],
                                start=True,
                                stop=True,
                            )
                        for j in range(2):
                            hp = j * 64
                            nc.tensor.matmul(
                                sums2[j][:, st, 1:2],
                                lhsT=z_sb[hp : hp + 64, st4 * 128 : (st4 + 1) * 128],
                                rhs=dummy[hp : hp + 64, :],
                                start=True,
                                stop=True,
                            )
                for j in range(2):
                    h = hq * 2 + j
                    hp = j * 64
                    sums_ps = sums2[j]
                    sums_sb = sm.tile([128, N_ST, 2], F32, tag="sums", bufs=2)
                    nc.vector.tensor_copy(out=sums_sb, in_=sums_ps)
                    m2 = sm.tile([128, N_ST], F32, tag="m2", bufs=2)
                    nc.vector.tensor_tensor(
                        out=m2, in0=sums_sb[:, :, 0], in1=sums_sb[:, :, 0],
                        op=mybir.AluOpType.mult,
                    )
                    nc.vector.tensor_scalar_mul(m2, m2, -1.0 / S)
                    nc.vector.tensor_tensor(
                        out=m2, in0=m2, in1=sums_sb[:, :, 1], op=mybir.AluOpType.add
                    )
                    # c = gamma / sqrt(M2/(S-1) + D*eps)
                    c8 = sm.tile([128, N_ST], F32, tag=f"c8_{h}", bufs=1)
                    nc.scalar.activation(
                        out=c8, in_=m2, func=AF.Sqrt, bias=eps_a, scale=1.0 / (S - 1)
                    )
                    nc.vector.reciprocal(out=c8, in_=c8)
                    nc.vector.tensor_scalar_mul(c8, c8, ga_bc[h])
                    if dbg:
                        nc.sync.dma_start(out=dc8[h], in_=c8)
                    c8_t[h] = c8

            pending = []

            def flush_pending():
                # deferred per-head normalization: by now the rrow reciprocal
                # has long finished, so the rbc matmul never stalls the PE
                while pending:
                    dst_hq, dst_sh, j, av_ps, rrow = pending.pop(0)
                    hp = j * 64
                    rbc_ps = ps.tile([128, 512], F32, tag="e", bufs=4)
                    nc.tensor.matmul(rbc_ps, lhsT=ones1, rhs=rrow, start=True, stop=True)
                    rbc_sb = sm.tile([128, 512], F32, tag="rbc", bufs=2)
                    nc.vector.tensor_copy(out=rbc_sb, in_=rbc_ps)
                    nc.vector.tensor_tensor(
                        out=aT[dst_hq][hp : hp + 64, dst_sh * 512 : (dst_sh + 1) * 512],
                        in0=av_ps[0:DHD, :],
                        in1=rbc_sb[0:DHD, :],
                        op=mybir.AluOpType.mult,
                    )

            for hq in range(4):
                qt_t = qT[hq]
                kt_t = kT[hq]
                for sh in range(N_SH):
                    pT2 = [
                        work.tile([128, 8, 512], FR, tag="pt16k", bufs=2, name=f"pT{j}")
                        for j in range(2)
                    ]
                    for stq in range(4):
                        st = sh * 4 + stq
                        # all 4 score matmuls adjacent: the two heads sit in
                        # distinct PE row groups and their half-bandwidth
                        # 64-partition rhs streams overlap
                        e_t = [[None, None], [None, None]]
                        for th in range(2):
                            for j in range(2):
                                hp = j * 64
                                eh = ps.tile([128, 512], F32, tag="e", bufs=4,
                                             name=f"eh{j}{th}")
                                e_t[j][th] = eh
                                nc.tensor.matmul(
                                    eh,
                                    lhsT=qt_t[hp : hp + 64, st * 128 : (st + 1) * 128],
                                    rhs=kt_t[hp : hp + 64, th * 512 : (th + 1) * 512],
                                    start=True,
                                    stop=True,
                                )
                        p2 = [None, None]
                        for j in range(2):
                            c8 = c8_t[hq * 2 + j]
                            p_sb = work.tile([128, S], FR, tag="big4k", bufs=14,
                                             name=f"p{j}")
                            p2[j] = p_sb
                            for th in range(2):
                                nc.scalar.activation(
                                    out=p_sb[:, th * 512 : (th + 1) * 512],
                                    in_=e_t[j][th], func=AF.Exp, bias=0.0,
                                    scale=c8[:, st : st + 1],
                                )
                        for j in range(2):
                            for half in range(2):
                                pt_ps = ps.tile([128, 4, 128], FR, tag="ptb", bufs=2)
                                for k in range(4):
                                    tj = half * 4 + k
                                    nc.tensor.transpose(
                                        pt_ps[:, k, :],
                                        p2[j][:, tj * 128 : (tj + 1) * 128],
                                        identr,
                                    )
                                nc.vector.tensor_copy(
                                    out=pT2[j][:, half * 4 : half * 4 + 4,
                                           stq * 128 : (stq + 1) * 128],
                                    in_=pt_ps,
                                )
                    av_list = []
                    for j in range(2):
                        h = hq * 2 + j
                        av_ps = ps.tile([DHD + 1, 512], F32, tag="pt", bufs=2)
                        for tj in range(8):
                            nc.tensor.matmul(
                                av_ps,
                                lhsT=v_sb[tj][:, h, :],
                                rhs=pT2[j][:, tj, :],
                                start=(tj == 0),
                                stop=(tj == 7),
                            )
                        rrow = sm.tile([1, 512], FR, tag="rrow", bufs=2)
                        with nc.allow_low_precision(reason="softmax denom 1/r in bf16"):
                            nc.vector.reciprocal(out=rrow, in_=av_ps[DHD : DHD + 1, :])
                        av_list.append((j, av_ps, rrow))
                    for j, av_ps, rrow in av_list:
                        pending.append((hq, sh, j, av_ps, rrow))
                    flush_pending()

            # ---------------- phase 3: WO + residual + LN1 ----------------
            x2_sb = [work.tile([128, D], F32, tag="v2k", bufs=10, name=f"x2_{i}") for i in range(N_ST)]
            x2T = [work.tile([128, S], FR, tag="big4k", bufs=14, name=f"x2T{i}") for i in range(3)]
            nc.vector.memset(x2T[2], 0.0)
            LCORR = float(D) / float(D - 1)

            def layer_norm(dst, src_ps, res_tiles, g_bc, b_bc):
                xr = sm.tile([128, D], F32, tag="xr", bufs=2)
                nc.vector.tensor_add(xr, src_ps, res_tiles[0])
                for rt in res_tiles[1:]:
                    nc.vector.tensor_add(xr, xr, rt)
                stats = sm.tile([128, 6], F32, tag="lstats", bufs=4)
                nc.vector.bn_stats(out=stats, in_=xr)
                mv = sm.tile([128, 2], F32, tag="lmv", bufs=4)
                nc.vector.bn_aggr(out=mv, in_=stats)
                sd = sm.tile([128, 1], F32, tag="lsd", bufs=4)
                nc.scalar.activation(
                    out=sd, in_=mv[:, 1:2], func=AF.Sqrt, bias=eps_l, scale=LCORR
                )
                rstd = sm.tile([128, 1], F32, tag="lrstd", bufs=4)
                nc.vector.reciprocal(out=rstd, in_=sd)
                grstd = sm.tile([128, 1], F32, tag="lgr", bufs=4)
                nc.vector.tensor_mul(grstd, rstd, g_bc)
                nc.vector.tensor_scalar(
                    out=dst,
                    in0=xr,
                    scalar1=mv[:, 0:1],
                    scalar2=grstd,
                    op0=mybir.AluOpType.subtract,
                    op1=mybir.AluOpType.mult,
                )
                nc.vector.tensor_scalar_add(dst, dst, b_bc)

            for st in range(N_ST):
                x1_ps = ps.tile([128, D], F32, tag="e", bufs=4)
                for it in range(4):
                    nc.tensor.matmul(
                        x1_ps,
                        lhsT=aT[it][:, st * 128 : (st + 1) * 128],
                        rhs=wo_sb[it],
                        start=(it == 0),
                        stop=(it == 3),
                    )
                layer_norm(x2_sb[st], x1_ps, [x_sb[:, st, :]], g1_bc, b1_bc)
                xt_ps = ps.tile([128, 4, 128], F32, tag="pt", bufs=2)
                for jc, (j0, jn) in enumerate([(0, 128), (128, 128), (256, 44)]):
                    nc.tensor.transpose(
                        xt_ps[:jn, jc, :], x2_sb[st][:, j0 : j0 + jn], ident
                    )
                for jc, (j0, jn) in enumerate([(0, 128), (128, 128), (256, 44)]):
                    nc.vector.tensor_copy(
                        out=x2T[jc][:jn, st * 128 : (st + 1) * 128],
                        in_=xt_ps[:jn, jc, :],
                    )

            # ---------------- phase 4: FFN + LN2 ----------------
            h1T = [work.tile([128, S], FR, tag="big4k", bufs=14, name=f"h1T{i}") for i in range(10)]
            for mt, (m0, msz) in enumerate(M_CHUNKS):
                for sh in range(N_SH):
                    h1_ps = ps.tile([128, 512], F32, tag="e", bufs=4)
                    for jc, (j0, jn) in enumerate(J_CHUNKS):
                        nc.tensor.matmul(
                            h1_ps[:msz, :],
                            lhsT=w1_sb[jc][:jn, m0 : m0 + msz],
                            rhs=x2T[jc][:jn, sh * 512 : (sh + 1) * 512],
                            start=(jc == 0),
                            stop=(jc == 2),
                        )
                    nc.scalar.activation(
                        out=h1T[mt][:msz, sh * 512 : (sh + 1) * 512],
                        in_=h1_ps[:msz, :],
                        func=AF.Relu,
                        bias=fb1_sb[mt][:msz, :],
                        scale=1.0,
                    )
            if dbg:
                for i in range(N_ST):
                    nc.sync.dma_start(out=dx2[i * 128 : (i + 1) * 128, :], in_=x2_sb[i])
                for mt, (m0, msz) in enumerate(M_CHUNKS):
                    mz = min(msz, DF - m0)
                    nc.sync.dma_start(out=dh1[m0 : m0 + mz, :], in_=h1T[mt][:mz, :].bitcast(F32))
            for st in range(N_ST):
                h2_ps = ps.tile([128, D], F32, tag="e", bufs=4)
                for mt, (m0, msz) in enumerate(M_CHUNKS):
                    nc.tensor.matmul(
                        h2_ps,
                        lhsT=h1T[mt][:msz, st * 128 : (st + 1) * 128],
                        rhs=w2_sb[mt][:msz, :],
                        start=(mt == 0),
                        stop=(mt == 9),
                    )
                o_sb = sm.tile([128, D], F32, tag="o", bufs=2)
                layer_norm(o_sb, h2_ps, [fb2_bc, x2_sb[st]], g2_bc, b2_bc)
                nc.sync.dma_start(out=outd[st * 128 : (st + 1) * 128, :], in_=o_sb)

    nc.compile()
    return nc


def _get_nc():
    if "nc" not in _cache:
        _cache["nc"] = _build_nc()
    return _cache["nc"]


def kernel(x, WQ, WK, WV, WO, W1, b1, W2, b2, gamma_a, beta_a,
           gamma1, beta1, gamma2, beta2):
    global _last_results
    import ml_dtypes

    f = np.float32
    bf = ml_dtypes.bfloat16
    x = np.asarray(x, f)

    def perm(W):
        # head h -> contiguous rows [h*64, (h+1)*64)
        return np.asarray(W, f).reshape(DHD, H, D).transpose(1, 0, 2).reshape(DH, D)

    def padr(a, rows, cols=None):
        out = np.zeros((rows, cols or a.shape[1]), f)
        out[: a.shape[0], : a.shape[1]] = a
        return out

    wq_t = padr(perm(WQ).T, DP).astype(bf)
    wk_t = padr(perm(WK).T, DP).astype(bf)
    wv_t = padr(perm(WV).T, DP).astype(bf)
    wo = np.ascontiguousarray(np.asarray(WO, f)).astype(bf)
    w1 = padr(np.asarray(W1, f), DP, DFP).astype(bf)
    w2 = padr(np.asarray(W2, f), DFP).astype(bf)
    fb1 = np.zeros((1280, 1), f)
    fb1[:DF, 0] = np.asarray(b1, f)
    fb2 = np.ascontiguousarray(np.asarray(b2, f))
    # beta_a drops out of softmax (per-row constant shift); the 1/sqrt(D)
    # score scale cancels inside the score LayerNorm: softmax(g*LN(e/sqrt(D)))
    # == softmax(g/sqrt(var(e) + D*eps) * e), so gamma is used unscaled and
    # D*eps replaces eps on-device.
    ga = np.ascontiguousarray(np.asarray(gamma_a, f).reshape(H, 1))
    ln = np.array(
        [np.asarray(gamma1, f), np.asarray(beta1, f),
         np.asarray(gamma2, f), np.asarray(beta2, f)], f
    ).reshape(4, 1)

    shared = {"wq": wq_t, "wk": wk_t, "wv": wv_t, "wo": wo, "w1": w1, "w2": w2,
              "fb1": fb1, "fb2": fb2, "ga": ga, "ln": ln}
    in_maps = []
    for b in range(B):
        xb = np.ascontiguousarray(x[b])
        in_maps.append({"x": xb,
                        "xt": padr(np.ascontiguousarray(xb.T), DP).astype(bf),
                        **shared})

    nc = _get_nc()
    res = run_bass_kernel_spmd(nc, in_maps, core_ids=list(range(NCORES)), trace=TRACE)
    _last_results = res
    return np.stack([res.results[b]["out"] for b in range(B)], axis=0)

